# revision 75
# baseline (speedup 1.0000x reference)
"""Causal single-head attention layer on 8 TRN2 NeuronCores.

Problem: X[4,2048,1024]; Q/K/V = X@W+b; scores = Q@K^T (no 1/sqrt(d));
causal mask; softmax; out = P@V.

Sharding: 2 cores per batch. Each core owns 8 query tiles (128 rows) of
its batch, folded for causal load balance:
  core h=0 -> global q-tiles (0,3,4,7,8,11,12,15)
  core h=1 -> global q-tiles (1,2,5,6,9,10,13,14)
Slot s on either core has causal extent <= 2s+2 k-tiles, so ONE uniform
program runs on all 8 cores; the exact causal boundary is a host-supplied
0/1 mask over the last two k-tiles of each slot.

Math restructuring:
  scores = (XqWq+bq)(XkWk+bk)^T
         = Xq G Xk^T + [q-only term] + w[k] + [const],  G = Wq Wk^T (host)
  q-only and const terms cancel in softmax; w[k] = Xk @ (Wk bq) (host)
  rides the per-partition bias slot of the Exp activation.
  The V projection is REASSOCIATED past the attention sum (q-extent 1024
  is half the k-extent 2048, so projecting after halves the PE rows):
    out = (E^T (Xk Wv))/rs + bv  ->  out = ((E^T Xk) Wv)/rs + bv
  No V tensor is ever formed; T^T[d,q] = sum_k X[k,d] E[k,q] accumulates
  per d-tile, then U = T@Wv per q-slot.

On-device layout (contraction always on partitions):
  The whole score path (xkt/xqt/g/qg) is fp16: halves the input stream
  vs f32, fp16 matmuls run 1 cyc/row with no fp32r moving>=256
  restriction (chains narrow to N=128), and its 10-bit mantissa keeps
  score error ~1e-2 absolute. Qg^T[d2,q] = G-proj of Xq^T; scores^T
  [k,q] accumulate fp32 in PSUM; E = exp(scores^T + w[k]) in bf16;
  T^T from xn
  (X natural layout, bf16) and E; U from T^T/wv (bf16); row sums via
  1-wide matmul with ones; out[q,d] = U/sums + bv.  No max-subtraction:
  |scores| <= ~60 so exp stays in fp32/bf16 range.

Scheduling (the limiting resources are the PE at 1 row/cycle and the
HWDGE descriptor generator at ~625ns per DMA — so DMAs are few and
big: whole 128-partition stripes, mask packed into 2 transfers):
  1. Qg is computed in dd-interleaved waves so real matmuls track the
     (g,xq) stripe pairs as they land: wave 1 = chains (do 0-3, q-lo)
     with dd outermost (supply-paced from ~4us), wave 2 = the 8 chains
     unlocked by the hi-half stripes, wave 3 = (do 4-7, q-hi). A short
     discarded warm-up chain covers the first pair's latency.
  2. Remaining inputs stream in exact consumption order: xkt-lo, msk,
     xn-lo, bvp, wv, xkt-hi, xn-hi.
  3. Per attention block: scores -> exp (+boundary mask on the idle
     Pool engine) -> T^T per d-tile -> per slot: rowsum, U, fused
     normalize+bias (scalar_tensor_tensor), DMA out. PSUM pool-open
     order is chosen so the first score chains land on banks the Qg
     phase vacated early.
"""

import numpy as np
import ml_dtypes

import concourse.bass as bass  # noqa: F401
import concourse.mybir as mybir
from concourse import bacc
from concourse.bass_utils import run_bass_kernel_spmd
from concourse.tile import TileContext

F32 = mybir.dt.float32
F32R = mybir.dt.float32r
BF16 = mybir.dt.bfloat16
F16 = mybir.dt.float16
EXP = mybir.ActivationFunctionType.Exp

B, S, D = 4, 2048, 1024
P = 128
DT = D // P          # 8 d-tiles
QT = 8               # q-tile slots per core
KT = S // P          # 16 k-tiles
EXT = [2 * s + 2 for s in range(QT)]   # uniform per-slot k-extent
BLK = [(0, 4, 8), (4, 8, 16)]          # (slot_lo, slot_hi, block k-extent)
WARMUP = 8                            # discarded ramp-fill matmuls (mid p-state)

QTS = {0: [0, 3, 4, 7, 8, 11, 12, 15], 1: [1, 2, 5, 6, 9, 10, 13, 14]}

_CACHE = {}


def _sc_off(kt, s0):
    # scores narrowing: slots below kt//2 never read k-tile kt. The
    # score path is fp16 (no fp32r moving>=256 restriction), so chains
    # narrow fully down to N=128.
    return min(max(0, (kt // 2 - s0)) * P, 384)


def _tt_off(kt, s0):
    # T^T accumulation is bf16 (no N>=256 penalty): cut down to N=128.
    return min(max(0, (kt // 2 - s0)) * P, 384)


def _build(reps=1):
    nc = bacc.Bacc("TRN2", target_bir_lowering=False, debug=False, num_devices=8)
    xqt = nc.declare_dram_parameter("xqt", [D, QT * P], F16, isOutput=False)
    xkt = nc.declare_dram_parameter("xkt", [D, S], F16, isOutput=False)
    xn = nc.declare_dram_parameter("xn", [S, D], BF16, isOutput=False)
    g = nc.declare_dram_parameter("g", [D, D], F16, isOutput=False)
    wv = nc.declare_dram_parameter("wv", [D, D], BF16, isOutput=False)
    wb = nc.declare_dram_parameter("wb", [P, KT], F32, isOutput=False)
    bvp = nc.declare_dram_parameter("bvp", [P, D], F32, isOutput=False)
    msk = nc.declare_dram_parameter("msk", [2, P, QT * P], BF16, isOutput=False)
    y = nc.declare_dram_parameter("y", [QT * P, D], F32, isOutput=True)

    with TileContext(nc) as tc:
      for _rep in range(reps):
        with tc.tile_pool(name="persist", bufs=1) as pp:
            # ---- persistent tiles ----
            xk_sb = [pp.tile([P, S], F16, tag=f"xk{i}", name=f"xk{i}") for i in range(DT)]
            xnlo_sb = [pp.tile([P, D], BF16, tag=f"xn{i}", name=f"xn{i}") for i in range(8)]
            qg_sb = [pp.tile([P, QT * P], F16, tag=f"qg{i}", name=f"qg{i}") for i in range(DT)]
            wv_sb = [pp.tile([P, D], BF16, tag=f"wvx{i}", name=f"wvx{i}") for i in range(DT)]
            wb_sb = pp.tile([P, KT], F32, tag="wb", name="wb")
            bv_sb = pp.tile([P, D], F32, tag="bv", name="bv")
            mask_sb = pp.tile([P, 2 * QT * P], BF16, tag="mask", name="mask")
            ones_sb = pp.tile([P, 1], BF16, tag="ones", name="ones")
            onesf_sb = pp.tile([P, 1], F32, tag="onesf", name="onesf")
            scr_sb = pp.tile([P, 1], F32, tag="scr", name="scr")
            warm_sb = pp.tile([P, 640], BF16, tag="warm", name="warm")

            nc.gpsimd.memset(warm_sb[:], 0.0)
            nc.gpsimd.memset(ones_sb[:], 1.0)
            nc.gpsimd.memset(onesf_sb[:], 1.0)
            # preload the Exp activation table while the PE warms up
            nc.scalar.activation(scr_sb[:], onesf_sb[:], EXP)

            def _dma_xkt_cols(c0, c1):
                for dd in range(DT):
                    nc.sync.dma_start(
                        out=xk_sb[dd][:, c0:c1],
                        in_=xkt[dd * P:(dd + 1) * P, c0:c1])

            # ---- Phase Qg: Qg^T[d2,q] = sum_d1 G[d1,d2] Xq^T[d1,q] ----
            with (
                tc.tile_pool(name="qgpool", bufs=1) as qp,
                tc.tile_pool(name="psq", bufs=8, space="PSUM") as ps,
            ):
                xq_sb = [qp.tile([P, QT * P], F16, tag=f"xq{i}", bufs=1,
                                 name=f"xq{i}") for i in range(DT)]
                g_sb = [qp.tile([P, D], F16, tag=f"gs{i}", bufs=1,
                                name=f"gs{i}") for i in range(DT)]

                # DMA order = consumption order; stripes are halved so the
                # first chains go as soon as ~4MB lands (the HWDGE
                # descriptor generator is a serial ~625ns/DMA device, so
                # DMAs are few and big).
                nc.sync.dma_start(out=wb_sb[:], in_=wb[:])
                for dd in range(DT):
                    nc.sync.dma_start(out=g_sb[dd][:, 0:512],
                                      in_=g[dd * P:(dd + 1) * P, 0:512])
                    nc.sync.dma_start(out=xq_sb[dd][:, 0:512],
                                      in_=xqt[dd * P:(dd + 1) * P, 0:512])
                for dd in range(DT):
                    nc.sync.dma_start(out=g_sb[dd][:, 512:1024],
                                      in_=g[dd * P:(dd + 1) * P, 512:1024])
                    nc.sync.dma_start(out=xq_sb[dd][:, 512:1024],
                                      in_=xqt[dd * P:(dd + 1) * P, 512:1024])
                _dma_xkt_cols(0, 1024)
                for j in range(2):
                    nc.sync.dma_start(
                        out=mask_sb[:, j * QT * P:(j + 1) * QT * P],
                        in_=msk[j])
                for kk in range(8):
                    nc.sync.dma_start(out=xnlo_sb[kk][:],
                                      in_=xn[kk * P:(kk + 1) * P, :])
                nc.sync.dma_start(out=bv_sb[:], in_=bvp[:])
                for dd in range(DT):
                    nc.sync.dma_start(out=wv_sb[dd][:],
                                      in_=wv[dd * P:(dd + 1) * P, :])
                _dma_xkt_cols(1024, 2048)

                # short warm-up covers the first dd-pair's DMA latency
                pw = ps.tile([P, 512], F32, tag="pq")
                for i in range(WARMUP):
                    nc.tensor.matmul(pw[:], warm_sb[:, 0:P], warm_sb[:, P:P + 512],
                                     start=(i == 0), stop=(i == WARMUP - 1))

                # Qg chains interleaved at dd (contraction) granularity so
                # real compute tracks the DMA stream pair-by-pair: wave 1
                # is supply-paced, waves 2-3 run under full supply.
                pqt = {}
                for do in range(4):
                    pqt[(do, 0)] = ps.tile([P, 512], F32, tag="pq", name=f"pqw{do}_0")
                for dd in range(DT):
                    for do in range(4):
                        nc.tensor.matmul(
                            pqt[(do, 0)][:],
                            g_sb[dd][:, do * P:(do + 1) * P],
                            xq_sb[dd][:, 0:512],
                            start=(dd == 0), stop=(dd == DT - 1),
                        )
                for do in range(4):
                    nc.vector.tensor_copy(qg_sb[do][:, 0:512], pqt[(do, 0)][:])

                for do in range(4, 8):
                    pqt[(do, 0)] = ps.tile([P, 512], F32, tag="pq", name=f"pqw{do}_0")
                for do in range(4):
                    pqt[(do, 1)] = ps.tile([P, 512], F32, tag="pq", name=f"pqw{do}_1")
                for dd in range(DT):
                    for do in range(4, 8):
                        nc.tensor.matmul(
                            pqt[(do, 0)][:],
                            g_sb[dd][:, do * P:(do + 1) * P],
                            xq_sb[dd][:, 0:512],
                            start=(dd == 0), stop=(dd == DT - 1),
                        )
                    for do in range(4):
                        nc.tensor.matmul(
                            pqt[(do, 1)][:],
                            g_sb[dd][:, do * P:(do + 1) * P],
                            xq_sb[dd][:, 512:1024],
                            start=(dd == 0), stop=(dd == DT - 1),
                        )
                for do in range(4, 8):
                    nc.vector.tensor_copy(qg_sb[do][:, 0:512], pqt[(do, 0)][:])
                for do in range(4):
                    nc.vector.tensor_copy(qg_sb[do][:, 512:1024], pqt[(do, 1)][:])

                for do in range(4, 8):
                    pq = ps.tile([P, 512], F32, tag="pq")
                    for dd in range(DT):
                        nc.tensor.matmul(
                            pq[:],
                            g_sb[dd][:, do * P:(do + 1) * P],
                            xq_sb[dd][:, 512:1024],
                            start=(dd == 0), stop=(dd == DT - 1),
                        )
                    nc.vector.tensor_copy(qg_sb[do][:, 512:1024], pq[:])
                # dummy allocation: takes the next PSUM rotation slot so the
                # first attention tile lands on an early-freed bank instead
                # of the one the final chain's copy is still reading.
                for dmy in range(2):
                    pqd = ps.tile([P, 512], F32, tag="pq", name=f"pqd{dmy}")
                    nc.tensor.matmul(pqd[:, 0:1], warm_sb[:, 0:P],
                                     warm_sb[:, P:P + 1], start=True, stop=True)


            # ---- Attention ----
            with (
                tc.tile_pool(name="estage", bufs=16) as ep,
                tc.tile_pool(name="xnhi", bufs=1) as xp,
                tc.tile_pool(name="tstage", bufs=1) as tp_,
                tc.tile_pool(name="ostage", bufs=2) as op,
                tc.tile_pool(name="small", bufs=4) as sp,
                # pool-open order controls PSUM bank assignment: pools whose
                # first use comes later (psu/pstt/psm) take the banks the
                # tail Qg chains just vacated; pssc gets banks that have
                # been free since early in the Qg phase, so the first score
                # chain never waits on the final Qg copy.
                tc.tile_pool(name="psu", bufs=2, space="PSUM") as ps_u,
                tc.tile_pool(name="pstt", bufs=2, space="PSUM") as ps_t,
                tc.tile_pool(name="psm", bufs=1, space="PSUM") as ps_m,
                tc.tile_pool(name="pssc", bufs=3, space="PSUM") as ps_s,
            ):
                xnhi_sb = [xp.tile([P, D], BF16, tag=f"xh{i}", bufs=1,
                                   name=f"xh{i}") for i in range(8)]
                for kk in range(8):
                    nc.sync.dma_start(out=xnhi_sb[kk][:],
                                      in_=xn[(8 + kk) * P:(9 + kk) * P, :])
                xn_sb = xnlo_sb + xnhi_sb

                for bi, (s0, s1, bext) in enumerate(BLK):
                    q0 = s0 * P
                    e_tiles = []
                    e_offs = []
                    # -- scores + exp, one chain per k-tile --
                    for kt in range(bext):
                        off = _sc_off(kt, s0)
                        n = 512 - off
                        pscore = ps_s.tile([P, 512], F32, tag="sc")
                        for dd in range(DT):
                            nc.tensor.matmul(
                                pscore[:, 0:n],
                                xk_sb[dd][:, kt * P:(kt + 1) * P],
                                qg_sb[dd][:, q0 + off:q0 + 512],
                                start=(dd == 0), stop=(dd == DT - 1),
                            )
                        et = ep.tile([P, 512], BF16, tag="E")
                        # E = exp(scores^T + w[k])  (w rides the bias slot)
                        nc.scalar.activation(et[:, 0:n], pscore[:, 0:n], EXP,
                                             bias=wb_sb[:, kt:kt + 1])
                        e_tiles.append(et)
                        e_offs.append(off)
                        # causal boundary mask: global slot ls = kt//2 owns
                        # k-tiles (2ls, 2ls+1); Pool engine, it's idle.
                        ls = kt // 2
                        if s0 <= ls < s1:
                            lo = (ls - s0) * P - off
                            j = kt % 2
                            nc.gpsimd.tensor_mul(
                                et[:, lo:lo + P],
                                et[:, lo:lo + P],
                                mask_sb[:, (j * QT + ls) * P:(j * QT + ls + 1) * P],
                            )

                    # -- T^T[d,q] = sum_k X[k,d] E[k,q], one chain per d-tile --
                    tt_sb = [tp_.tile([P, 512], BF16, tag=f"tt{i}", bufs=1,
                                      name=f"tt{bi}_{i}") for i in range(DT)]
                    for dt in range(DT):
                        ptt = ps_t.tile([P, 512], F32, tag="tt")
                        for kt in range(bext):
                            toff = _tt_off(kt, s0)
                            nc.tensor.matmul(
                                ptt[:, toff:512],
                                xn_sb[kt][:, dt * P:(dt + 1) * P],
                                e_tiles[kt][:, toff - e_offs[kt]:512 - e_offs[kt]],
                                start=(kt == 0), stop=(kt == bext - 1),
                            )
                        nc.vector.tensor_copy(tt_sb[dt][:], ptt[:])

                    # -- U[q,d2] = sum_d T^T[d,q]^T Wv[d,d2], per slot --
                    for ls in range(s0, s1):
                        lq = (ls - s0) * P
                        ext = EXT[ls]
                        # rowsum via 1-wide matmul chain (~free on PE)
                        pm = ps_m.tile([P, 1], F32, tag="pm")
                        for kt in range(ext):
                            el = e_tiles[kt][:, lq - e_offs[kt]:lq - e_offs[kt] + P]
                            nc.tensor.matmul(pm[:], el, ones_sb[:],
                                             start=(kt == 0), stop=(kt == ext - 1))
                        rc = sp.tile([P, 1], F32, tag="rc")
                        nc.vector.reciprocal(rc[:], pm[:])
                        ot = op.tile([P, D], F32, tag="ot")
                        # final slot: 256-wide chunks + separate flushes so
                        # the after-last-matmul drain tail is short
                        nchunk = 4 if ls == QT - 1 else 2
                        cw = D // nchunk
                        for c in range(nchunk):
                            pu = ps_u.tile([P, cw], F32, tag="pu")
                            for dt in range(DT):
                                nc.tensor.matmul(
                                    pu[:],
                                    tt_sb[dt][:, lq:lq + P],
                                    wv_sb[dt][:, c * cw:(c + 1) * cw],
                                    start=(dt == 0), stop=(dt == DT - 1),
                                )
                            # fused (pu * 1/rowsum) + bv in one DVE op
                            nc.vector.scalar_tensor_tensor(
                                ot[:, c * cw:(c + 1) * cw], pu[:], rc[:],
                                bv_sb[:, c * cw:(c + 1) * cw],
                                mybir.AluOpType.mult, mybir.AluOpType.add)
                            if ls == QT - 1:
                                nc.sync.dma_start(
                                    out=y[ls * P:(ls + 1) * P,
                                          c * cw:(c + 1) * cw],
                                    in_=ot[:, c * cw:(c + 1) * cw])
                        if ls != QT - 1:
                            nc.sync.dma_start(out=y[ls * P:(ls + 1) * P, :],
                                              in_=ot[:])

    nc.compile()
    return nc


def _get_nc():
    if "nc" not in _CACHE:
        _CACHE["nc"] = _build()
    return _CACHE["nc"]


def make_in_maps(X, Wq, bq, Wk, bk, Wv, bv):
    X = np.asarray(X, np.float32)
    Wq = np.asarray(Wq, np.float32)
    Wk = np.asarray(Wk, np.float32)
    Wv = np.ascontiguousarray(np.asarray(Wv, np.float32))
    bq = np.asarray(bq, np.float32)
    bv = np.asarray(bv, np.float32)

    G = np.ascontiguousarray(Wq @ Wk.T).astype(np.float16)   # [D, D]
    wkbq = Wk @ bq                               # [D]
    bvp = np.ascontiguousarray(np.broadcast_to(bv[None, :], (P, D)))
    wv_bf = Wv.astype(ml_dtypes.bfloat16)

    # boundary masks packed [j, P(k-sub), slot*P(q)] so each j half is
    # one DMA: m[j, kk, s*P+qq] = (2s+j)*P+kk <= qtile(s)*P+qq
    masks = {}
    for h in (0, 1):
        m = np.zeros((2, P, QT * P), np.float32)
        for s in range(QT):
            qt = QTS[h][s]
            qq = qt * P + np.arange(P)[None, :]
            for j in range(2):
                kk = (2 * s + j) * P + np.arange(P)[:, None]
                m[j, :, s * P:(s + 1) * P] = (kk <= qq)
        masks[h] = m.astype(ml_dtypes.bfloat16)

    in_maps = []
    for c in range(8):
        b, h = divmod(c, 2)
        Xb = X[b]
        xkt = np.ascontiguousarray(Xb.T).astype(np.float16)
        xq_rows = np.concatenate(
            [Xb[qt * P:(qt + 1) * P] for qt in QTS[h]], axis=0)
        xqt = np.ascontiguousarray(xq_rows.T).astype(np.float16)
        w = Xb @ wkbq                             # [S] additive k-bias
        wbp = np.ascontiguousarray(w.reshape(KT, P).T)   # [P, KT]
        in_maps.append({
            "xqt": xqt, "xkt": xkt, "xn": Xb.astype(ml_dtypes.bfloat16),
            "g": G, "wv": wv_bf,
            "wb": wbp, "bvp": bvp, "msk": masks[h],
        })
    return in_maps


def assemble(results):
    Y = np.empty((B, S, D), np.float32)
    for c in range(8):
        b, h = divmod(c, 2)
        yc = results[c]["y"]
        for s in range(QT):
            qt = QTS[h][s]
            Y[b, qt * P:(qt + 1) * P, :] = yc[s * P:(s + 1) * P, :]
    return Y


def kernel(X, Wq, bq, Wk, bk, Wv, bv):
    nc = _get_nc()
    in_maps = make_in_maps(X, Wq, bq, Wk, bk, Wv, bv)
    res = run_bass_kernel_spmd(nc, in_maps, core_ids=list(range(8)))
    return assemble(res.results)
